# revision 1
# baseline (speedup 1.0000x reference)
"""COIL sparse-attention scoring kernel for 8 Trainium2 NeuronCores.

Strategy
--------
Shard the doc axis (Bd=128) across the 8 cores (16 docs each); qry tensors are
replicated. The exact-token-match mask is folded INTO the matmul: each token id
(vocab 1000) is encoded as three base-10 digit one-hots scaled by ALPHA=32 and
appended to the reps. Then

    v[qs, ct] = <qry_ext[qs], doc_ext[ct]> = S[qs, ct] + 1024 * match_digits

where match_digits == 3 iff the ids are equal, so

    tok[qs, c] = relu(max_t v[qs, c, t] - 3072)

reproduces the reference masked-max exactly (non-match scores sit below 2100,
matches above 3000). The qry reps are split hi/lo in bf16 (3 cross terms) so
the matmul runs at full bf16 rate with ~fp32 accuracy: K = 3*32 + 30 = 126.

Per core: 32 q-tiles of 128 q-positions; each q-tile is one [126,128]x[126,2048]
matmul into PSUM [128, 2048]. The per-doc max over the 128 doc tokens is split
between the DVE (direct tensor_reduce from PSUM) and a ScalarE relu-convert to
fp16 followed by a DVE tensor_tensor max tree at 2x rate. The sum over query
positions is a ones-vector matmul on the PE (partition-dim sum). CLS scores,
the tiny 4-way q-tile fold, and the final max over the 8 query chunks are done
on host (a few thousand elements).
"""

import os
import numpy as np
import ml_dtypes

Bq, Sq, Bd, Sd, D, Dc = 8, 512, 128, 128, 32, 768
NCORES = 8
BD_PER = Bd // NCORES          # 16 docs per core
K_EXT = 126                    # 32*3 rep dims + 30 one-hot dims
SQF = Bq * Sq                  # 4096 query positions
DCOL = BD_PER * Sd             # 2048 doc tokens per core
NQT = SQF // 128               # 32 q-tiles
ALPHA = 32.0
OFF = 3.0 * ALPHA * ALPHA      # 3072: offset of a full 3-digit match
# q-tile qt goes to the DVE-direct path iff qt % DIRECT_PERIOD == DIRECT_PERIOD-1;
# the rest go ScalarE-relu-fp16 -> DVE max tree. Whole-tile assignment keeps each
# PSUM tile single-reader (fewer semaphore waits).
DIRECT_PERIOD = int(os.environ.get("KERNEL_DIRECT_PERIOD", "4"))
TREE_LEVELS = int(os.environ.get("KERNEL_TREE_LEVELS", "3"))
# 6 warm-up MMs (~3.8us busy) sits right at the HAM 3.4us flip threshold and
# is bimodal run-to-run (71.6 vs 75.5 measured); 12 is ~0.5us slower at best
# but stable across runs.
WARMUP_MMS = int(os.environ.get("KERNEL_WARMUP_MMS", "12"))
BRIDGE_MMS = int(os.environ.get("KERNEL_BRIDGE_MMS", "0"))
# prune query positions whose token id does not appear in this core's doc
# slab (they contribute exactly 0): compact to NQT_PRUNED q-tiles per core
PRUNE = os.environ.get("KERNEL_PRUNE", "1") == "1"
NQT_PRUNED = int(os.environ.get("KERNEL_NQT_PRUNED", "29"))
# fraction knob: tree q-tiles where GPSIMD runs the first max-tree level
# instead of the DVE. Disabled: this walrus build rejects TensorTensor on
# the Pool engine ("Instruction engine check failed").
GPSIMD_TT1_MOD = int(os.environ.get("KERNEL_GPSIMD_TT1_MOD", "0"))

_CACHE = {}


def _bf16(x):
    return x.astype(ml_dtypes.bfloat16)


def _onehot_digits(ids):
    """ids [N] int in [0,1000) -> [N,30] base-10 digit one-hot (float32)."""
    n = ids.shape[0]
    H = np.zeros((n, 30), dtype=np.float32)
    r = np.arange(n)
    H[r, ids % 10] = 1.0
    H[r, 10 + (ids // 10) % 10] = 1.0
    H[r, 20 + ids // 100] = 1.0
    return H


def _build_qry_ext(qry_reps, qry_input_ids, qry_attention_mask):
    q = np.asarray(qry_reps, np.float32).reshape(SQF, D)
    ids = np.asarray(qry_input_ids, np.int64).reshape(SQF)
    q_hi = _bf16(q).astype(np.float32)
    q_lo = _bf16(q - q_hi).astype(np.float32)
    H = ALPHA * _onehot_digits(ids)
    ext = np.concatenate([q_hi, q_lo, q_hi, H], axis=1)  # [SQF, 126]
    # rows that must contribute 0: CLS (s=0), SEP (last attended pos), mask==0
    mask = np.asarray(qry_attention_mask, np.int64).copy()
    sep = mask.sum(axis=1) - 1
    mask[np.arange(Bq), sep] = 0
    mask[:, 0] = 0
    ext *= mask.reshape(SQF, 1).astype(np.float32)
    return np.ascontiguousarray(_bf16(ext).T)  # [126, SQF]


def _build_doc_ext(doc_reps, doc_input_ids):
    d = np.asarray(doc_reps, np.float32).reshape(-1, D)
    ids = np.asarray(doc_input_ids, np.int64).reshape(-1)
    d_hi = _bf16(d).astype(np.float32)
    d_lo = _bf16(d - d_hi).astype(np.float32)
    H = ALPHA * _onehot_digits(ids)
    ext = np.concatenate([d_hi, d_hi, d_lo, H], axis=1)  # [N, 126]
    return np.ascontiguousarray(_bf16(ext).T)  # [126, N]


_LDW_PATCHED = False


def _patch_ldw_opt():
    """bir_verify_and_optimise hardcodes --enable-ldw-opt=false, which makes
    walrus emit one LDWEIGHTS per matmul even when the stationary operand is
    unchanged (4x redundant here). Append =true (last flag wins)."""
    global _LDW_PATCHED
    # Tile emits standalone InstLdweights, which walrus's ldw-opt rejects;
    # keep this opt-in for experiments only.
    if _LDW_PATCHED or not os.environ.get("KERNEL_LDW_OPT"):
        return
    import concourse.bass_utils as bu

    orig = bu.get_walrus_args

    def patched(*a, **k):
        return orig(*a, **k) + ["--enable-ldw-opt=true"]

    bu.get_walrus_args = patched
    _LDW_PATCHED = True


def _split_multi_waits(nc, mybir):
    """This container's walrus accepts only ONE sync-wait per instruction
    ("Too many sync wait commands"). Hoist extra waits into standalone
    EventSemaphore instructions on the same engine right before the offender
    (the sequencer blocks on each in order — semantically identical)."""
    n = 0
    for func in nc.m.functions:
        for bb in func.blocks:
            out = []
            for inst in bb.instructions:
                si = inst.sync_info
                if si is not None and len(si.on_wait) > 1:
                    waits = list(si.on_wait)
                    for w in waits[:-1]:
                        n += 1
                        out.append(
                            mybir.InstEventSemaphore(
                                name=f"W-{inst.name}-{n}",
                                engine=inst.engine,
                                ins=[],
                                outs=[],
                                debug=inst.debug,
                                sync_info=mybir.SyncInfo(
                                    on_wait=[w], on_update=[]
                                ),
                            )
                        )
                    inst.sync_info = mybir.SyncInfo(
                        on_wait=[waits[-1]], on_update=list(si.on_update)
                    )
                out.append(inst)
            bb.instructions = out
    return n


def _groups(nqt):
    """Final-sum groups: up to 8 q-tiles share one selector matmul (the
    off-diagonal blocks of the [8G, 16G] product are computed but unused)."""
    return [range(g, min(g + 8, nqt)) for g in range(0, nqt, 8)]


def _build_nc(direct_period, tree_levels, nqt):
    import concourse.bass as bass
    import concourse.mybir as mybir
    import concourse.tile as tile
    from concourse.bass import ts

    bf16, f16, f32 = mybir.dt.bfloat16, mybir.dt.float16, mybir.dt.float32
    nc = bass.Bass("TRN2", target_bir_lowering=False, debug=False)
    sqf = nqt * 128
    qryT = nc.dram_tensor("qryT", [K_EXT, sqf], bf16, kind="ExternalInput").ap()
    docT = nc.dram_tensor("docT", [K_EXT, DCOL], bf16, kind="ExternalInput").ap()
    selT = nc.dram_tensor("selT", [128, 8 * nqt], f32, kind="ExternalInput").ap()
    out = nc.dram_tensor("out", [64, 16 * nqt], f32, kind="ExternalOutput").ap()

    phase = int(os.environ.get("KERNEL_DIRECT_PHASE", "0"))
    is_direct = [
        direct_period > 0 and qt % direct_period == phase % direct_period
        for qt in range(nqt)
    ]
    n_direct = sum(is_direct)
    with tile.TileContext(nc) as tc:
        with (
            tc.tile_pool(name="inp", bufs=1) as inp,
            tc.tile_pool(name="psum", bufs=2, space="PSUM") as psum,
            tc.tile_pool(name="stage", bufs=3) as stp,
            tc.tile_pool(name="tree", bufs=2) as trp,
            tc.tile_pool(name="accp", bufs=1) as accp,
        ):
            # PE warm-up: ~3.5us of junk matmuls during the DMA head so the
            # HAM clock-gate reaches 8/8 before the real work starts
            scratch = inp.tile([K_EXT, 512], bf16)
            nc.vector.memset(scratch[:], 0.0)
            wps = psum.tile([128, 512], f32, tag="score")
            for _ in range(WARMUP_MMS):
                nc.tensor.matmul(
                    wps[:], scratch[:, 0:128], scratch[:], start=True, stop=True
                )

            # doc chunk 0 + qry chunk 0 first so q-tile 0 can start early
            qry_sb = inp.tile([K_EXT, sqf], bf16)
            doc_sb = inp.tile([K_EXT, DCOL], bf16)
            sel_sb = inp.tile([128, 8 * nqt], f32)
            # first chunks split across the HWDGE (sync) and SWDGE (gpsimd)
            # queues so they land in parallel instead of serializing
            nc.sync.dma_start(doc_sb[:, ts(0, 512)], docT[:, ts(0, 512)])
            nc.gpsimd.dma_start(qry_sb[:, ts(0, 512)], qryT[:, ts(0, 512)])
            nc.sync.dma_start(doc_sb[:, ts(1, 512)], docT[:, ts(1, 512)])
            nc.gpsimd.dma_start(doc_sb[:, ts(2, 512)], docT[:, ts(2, 512)])
            nc.sync.dma_start(doc_sb[:, ts(3, 512)], docT[:, ts(3, 512)])
            for off in range(512, sqf, 512):
                w = min(512, sqf - off)
                nc.sync.dma_start(qry_sb[:, off : off + w], qryT[:, off : off + w])
            nc.sync.dma_start(sel_sb[:], selT[:])

            accum = accp.tile([128, 16 * nqt], f32)
            draw = accp.tile([128, 16 * max(n_direct, 1)], f32)
            negoff = accp.tile([128, 1], f32)
            nc.vector.memset(negoff[:], -OFF)

            di = 0
            for qt in range(nqt):
                ps = psum.tile([128, DCOL], f32, tag="score")
                for j in range(DCOL // 512):
                    nc.tensor.matmul(
                        ps[:, ts(j, 512)],
                        qry_sb[:, ts(qt, 128)],
                        doc_sb[:, ts(j, 512)],
                        start=True,
                        stop=True,
                    )
                if is_direct[qt]:
                    # whole tile on DVE straight from PSUM (raw v scale),
                    # then tok = max(raw, OFF) - OFF into the accum cols
                    nc.vector.reduce_max(
                        draw[:, di * 16 : (di + 1) * 16],
                        ps[:].rearrange("p (c t) -> p c t", t=Sd),
                        axis=mybir.AxisListType.X,
                    )
                    nc.vector.tensor_scalar(
                        accum[:, qt * 16 : (qt + 1) * 16],
                        draw[:, di * 16 : (di + 1) * 16],
                        OFF,
                        -OFF,
                        mybir.AluOpType.max,
                        mybir.AluOpType.add,
                    )
                    di += 1
                else:
                    # fp16 relu(v - OFF) on ScalarE; tree then maxes toks
                    st = stp.tile([128, BD_PER * Sd], f16, tag="stage")
                    nc.scalar.activation(
                        st[:],
                        ps[:],
                        mybir.ActivationFunctionType.Relu,
                        bias=negoff[:],
                    )
                    cur, width = st, Sd
                    for lev in range(tree_levels):
                        nxt = trp.tile([128, BD_PER * width // 2], f16, tag=f"t{lev}")
                        cv = cur[:].rearrange("p (c t) -> p c t", t=width)
                        eng = (
                            nc.gpsimd
                            if (
                                lev == 0
                                and GPSIMD_TT1_MOD > 0
                                and qt % GPSIMD_TT1_MOD == 0
                            )
                            else nc.vector
                        )
                        eng.tensor_max(
                            nxt[:].rearrange("p (c t) -> p c t", t=width // 2),
                            cv[:, :, 0 : width // 2],
                            cv[:, :, width // 2 : width],
                        )
                        cur, width = nxt, width // 2
                    nc.vector.reduce_max(
                        accum[:, qt * 16 : (qt + 1) * 16],
                        cur[:].rearrange("p (c t) -> p c t", t=width),
                        axis=mybir.AxisListType.X,
                    )
            # a few junk matmuls with late priority: the scheduler runs them
            # when the PE idles after the last q-tile, keeping the HAM clock
            # warm for the final partition-sum matmuls
            for _ in range(BRIDGE_MMS):
                bp = psum.tile([128, 512], f32, tag="score")
                nc.tensor.matmul(
                    bp[:], scratch[:, 0:128], scratch[:], start=True, stop=True
                )
            # per-q partition sums: for each group of up to 8 q-tiles, one
            # matmul with the q-membership selector as the stationary operand;
            # only the [8,16] diagonal blocks are used (host slices them out)
            osb = accp.tile([64, 16 * nqt], f32)
            nc.vector.memset(osb[:], 0.0)
            for g, grp in enumerate(_groups(nqt)):
                qts = list(grp)
                gn = len(qts)
                c0 = qts[0] * 16
                fin = psum.tile([8 * gn, 16 * gn], f32, tag="score")
                nc.tensor.matmul(
                    fin[:],
                    sel_sb[:, qts[0] * 8 : (qts[-1] + 1) * 8],
                    accum[:, c0 : c0 + 16 * gn],
                    start=True,
                    stop=True,
                )
                if g % 2 == 0:
                    nc.vector.tensor_copy(osb[0 : 8 * gn, c0 : c0 + 16 * gn], fin[:])
                else:
                    nc.scalar.copy(osb[0 : 8 * gn, c0 : c0 + 16 * gn], fin[:])
            nc.sync.dma_start(out[:], osb[:])
    _split_multi_waits(nc, mybir)
    return nc


def _get_nc(nqt):
    _patch_ldw_opt()
    key = (
        DIRECT_PERIOD,
        TREE_LEVELS,
        nqt,
        os.environ.get("KERNEL_DIRECT_PHASE", "0"),
    )
    if key not in _CACHE:
        _CACHE[key] = _build_nc(key[0], key[1], nqt)
    return _CACHE[key]


def _qry_row_mask(inputs):
    """[Bq, Sq] bool: rows that can contribute (attended, not CLS/SEP)."""
    mask = np.asarray(inputs["qry_attention_mask"], np.int64).copy()
    sep = mask.sum(axis=1) - 1
    mask[np.arange(Bq), sep] = 0
    mask[:, 0] = 0
    return mask.astype(bool)


def _prepare_in_maps(inputs):
    qT_full = _build_qry_ext(
        inputs["qry_reps"], inputs["qry_input_ids"], inputs["qry_attention_mask"]
    )  # [K_EXT, SQF] bf16
    doc_reps = np.asarray(inputs["doc_reps"], np.float32)
    doc_ids = np.asarray(inputs["doc_input_ids"], np.int64)
    qry_ids = np.asarray(inputs["qry_input_ids"], np.int64).reshape(SQF)
    row_ok = _qry_row_mask(inputs).reshape(SQF)
    qpos_q = np.repeat(np.arange(Bq), Sq)  # q index of each row

    nqt = NQT
    sels = None
    if PRUNE:
        sels = []
        for core in range(NCORES):
            sl = slice(core * BD_PER, (core + 1) * BD_PER)
            vocab = np.zeros(1000, dtype=bool)
            vocab[doc_ids[sl].reshape(-1)] = True
            keep = row_ok & vocab[qry_ids]
            sels.append(np.nonzero(keep)[0])
        if max(len(s) for s in sels) <= NQT_PRUNED * 128:
            nqt = NQT_PRUNED
        else:  # fallback: shapes must be compile-time fixed
            sels = None

    in_maps = []
    sqf = nqt * 128
    for core in range(NCORES):
        sl = slice(core * BD_PER, (core + 1) * BD_PER)
        dT = _build_doc_ext(doc_reps[sl], doc_ids[sl])
        if sels is not None:
            rows = sels[core]
            qT = np.zeros((K_EXT, sqf), dtype=qT_full.dtype)
            qT[:, : len(rows)] = qT_full[:, rows]
            qrow = qpos_q[rows]
        else:
            qT = qT_full
            qrow = qpos_q
        # selector: sel[p, qt*8+m] = 1 iff row qt*128+p belongs to query m
        sel = np.zeros((128, 8 * nqt), dtype=np.float32)
        for qt in range(nqt):
            seg = qrow[qt * 128 : (qt + 1) * 128]
            sel[np.arange(len(seg)), qt * 8 + seg] = 1.0
        in_maps.append({"qryT": qT, "docT": dT, "selT": sel})
    return in_maps, nqt


def _assemble(inputs, results, nqt):
    toks = np.zeros((Bq, Bd), dtype=np.float32)
    for core in range(NCORES):
        osb = np.asarray(results[core]["out"], np.float32)  # [64, 16*nqt]
        part = np.zeros((Bq, BD_PER), dtype=np.float32)
        for g, grp in enumerate(_groups(nqt)):
            for tl, qt in enumerate(grp):
                part += osb[8 * tl : 8 * tl + 8, qt * 16 : (qt + 1) * 16]
        toks[:, core * BD_PER : (core + 1) * BD_PER] = part
    cls = np.asarray(inputs["qry_cls"], np.float32) @ np.asarray(
        inputs["doc_cls"], np.float32
    ).T
    scores = toks + cls
    return scores.max(axis=0).reshape(-1).astype(np.float32)


def _ensure_ntff_hook():
    """This container's antenv lacks axon_hooks; synthesize the module and
    register the ctypes-based NTFF profile hook so trace=True works."""
    import sys
    import types

    if "antenv.axon_hooks" in sys.modules:
        return
    mod = types.ModuleType("antenv.axon_hooks")
    state = {"hook": None}
    mod.set_axon_ntff_profile_hook = lambda h: state.__setitem__("hook", h)
    mod.get_axon_ntff_profile_hook = lambda: state["hook"]
    sys.modules["antenv.axon_hooks"] = mod
    try:
        import antenv

        antenv.axon_hooks = mod
    except ImportError:
        pass
    try:
        from trn_agent_boot.trn_boot import _ntff_profile_via_ctypes

        mod.set_axon_ntff_profile_hook(
            _ntff_profile_via_ctypes("/opt/axon/libaxon_pjrt.so")
        )
    except Exception:
        pass


def run(inputs, trace=False, **kwargs):
    """Run on the 8 NeuronCores; returns (output, BassKernelResults)."""
    from concourse.bass_utils import run_bass_kernel_spmd

    if trace:
        _ensure_ntff_hook()
    in_maps, nqt = _prepare_in_maps(inputs)
    nc = _get_nc(nqt)
    res = run_bass_kernel_spmd(
        nc, in_maps, core_ids=list(range(NCORES)), trace=trace, **kwargs
    )
    return _assemble(inputs, res.results, nqt), res


def kernel(**inputs) -> np.ndarray:
    out, _ = run(inputs)
    return out



# revision 2
# speedup vs baseline: 1.0783x; 1.0783x over previous
"""COIL sparse-attention scoring kernel v2: per-doc q-compacted chunks.

Strategy
--------
Shard docs 16/core. A query row can only contribute to a doc containing its
token id (~12% of (q,doc) pairs; host knows this from ids alone). For each
(core, doc) the host compacts the matching query rows into 128-slot chunks;
the device runs one [62,128]x[62,128] bf16 matmul per chunk (K = 32 rep dims
+ 30 digit one-hot dims so a full id match adds +3072 to the score) and
reduces max over the doc's 128 tokens per slot. This cuts PSUM volume ~7x
vs the dense [all-q x all-doc-tokens] formulation.

Docs are sorted by need and slot-assigned so the per-slot chunk counts
(pattern) are consistent across cores. Chunks are grouped in slot pairs into
[128, cg*128] PSUM tiles. K=62 <= 64 lets two chunks run CONCURRENTLY as
row-tiled matmuls (tile_position (0,0) / (64,0)): even chunks live in SBUF
partitions 0-61, odd chunks in 64-125 (docT replicated to both halves), so
each LDWEIGHTS overlaps the other row-half's matmul and the PE streams at
~1 chunk per 107ns even cold. Groups are consumed either by DVE reduce_max
straight from PSUM (raw max; host relu-shifts) or ScalarE relu(v-OFF) into
fp16 staging with batched DVE reduce_max at 2x rate. Input/output DMA is
spread over the three DGE queues (sync/scalar/gpsimd, ~40-50 GB/s each) in
consumption order. Host scatters per-slot tok values back to [Bq, Bd], adds
cls scores, maxes over Bq.
"""

import os
import numpy as np
import ml_dtypes

Bq, Sq, Bd, Sd, D = 8, 512, 128, 128, 32
NCORES = 8
BD_PER = Bd // NCORES
SQF = Bq * Sq
# fp8 e3m4 inputs (4-bit mantissa, max 15.5): reps fit (|x|<5.2), digit
# one-hots at ALPHA=12 are exact, and input DMA bytes halve vs bf16 — the
# binding constraint is HBM bandwidth shared across all 8 cores (~40GB/s
# per core when all load simultaneously).
K_EXT = 56  # 32 rep dims + 24 base-6 digit one-hot dims
ALPHA = 12.0
NDIGITS = 4
OFF = NDIGITS * ALPHA * ALPHA  # 576
WARMUP_MMS = int(os.environ.get("KERNEL_WARMUP_MMS", "10"))
DIRECT_GROUPS = tuple(
    int(x) for x in os.environ.get("KERNEL_DIRECT_GROUPS", "3,7").split(",") if x != ""
)

_CACHE = {}


def _fp8(x):
    return x.astype(ml_dtypes.float8_e3m4)


def _onehot_digits(ids):
    """base-6 4-digit one-hot (ids < 1000 < 6^4): [N, 24]."""
    n = ids.shape[0]
    H = np.zeros((n, 24), dtype=np.float32)
    r = np.arange(n)
    H[r, ids % 6] = 1.0
    H[r, 6 + (ids // 6) % 6] = 1.0
    H[r, 12 + (ids // 36) % 6] = 1.0
    H[r, 18 + ids // 216] = 1.0
    return H


def _qry_row_mask(inputs):
    mask = np.asarray(inputs["qry_attention_mask"], np.int64).copy()
    sep = mask.sum(axis=1) - 1
    mask[np.arange(Bq), sep] = 0
    mask[:, 0] = 0
    return mask.astype(bool)


def _build_qry_ext(inputs):
    q = np.asarray(inputs["qry_reps"], np.float32).reshape(SQF, D)
    ids = np.asarray(inputs["qry_input_ids"], np.int64).reshape(SQF)
    ext = np.concatenate(
        [_fp8(q).astype(np.float32), ALPHA * _onehot_digits(ids)], axis=1
    )
    ext *= _qry_row_mask(inputs).reshape(SQF, 1)
    return np.ascontiguousarray(_fp8(ext).T)  # [56, SQF]


def _build_doc_ext(doc_reps, doc_ids):
    d = np.asarray(doc_reps, np.float32).reshape(-1, D)
    ids = np.asarray(doc_ids, np.int64).reshape(-1)
    ext = np.concatenate(
        [_fp8(d).astype(np.float32), ALPHA * _onehot_digits(ids)], axis=1
    )
    return np.ascontiguousarray(_fp8(ext).T)  # [56, N]


def _layout(pattern):
    """Chunk layout derived purely from the slot pattern (shared by host and
    device builder). Returns per-group dicts:
      chunks: list of (slot, k) in PSUM column order (evens then odds)
      ne/no: even/odd counts; e0/o0: column offsets into qchE/qchO
    plus global chunk-id order (group-major, psum order) for output mapping.
    """
    groups = []
    e_off = o_off = 0
    for g in range(BD_PER // 2):
        s0, s1 = 2 * g, 2 * g + 1
        seq = [(s0, k) for k in range(pattern[s0])] + [
            (s1, k) for k in range(pattern[s1])
        ]
        ev, od = seq[0::2], seq[1::2]
        groups.append(
            {
                "slots": (s0, s1),
                "ev": ev,
                "od": od,
                "e0": e_off,
                "o0": o_off,
                "cg": len(seq),
            }
        )
        e_off += len(ev)
        o_off += len(od)
    return groups, e_off, o_off


def prepare(inputs):
    qT = _build_qry_ext(inputs)
    doc_reps = np.asarray(inputs["doc_reps"], np.float32)
    doc_ids = np.asarray(inputs["doc_input_ids"], np.int64)
    qry_ids = np.asarray(inputs["qry_input_ids"], np.int64).reshape(SQF)
    row_ok = _qry_row_mask(inputs).reshape(SQF)
    qpos_b = np.repeat(np.arange(Bq), Sq)

    cores = []
    for core in range(NCORES):
        sl = slice(core * BD_PER, (core + 1) * BD_PER)
        ids_slab = doc_ids[sl]
        lists = []
        for d in range(BD_PER):
            vocab = np.zeros(1000, dtype=bool)
            vocab[ids_slab[d]] = True
            lists.append(np.nonzero(row_ok & vocab[qry_ids])[0])
        bysize = np.argsort([-len(L) for L in lists], kind="stable")
        order = np.empty(BD_PER, dtype=np.int64)
        order[0::2] = bysize[: BD_PER // 2]
        order[1::2] = bysize[BD_PER // 2 :][::-1]
        cores.append((sl, ids_slab, lists, order))

    pattern = tuple(
        max(max((len(c[2][c[3][i]]) + 127) // 128 for c in cores), 1)
        for i in range(BD_PER)
    )
    groups, TE, TO = _layout(pattern)

    in_maps, metas = [], []
    for core in range(NCORES):
        sl, ids_slab, lists, order = cores[core]
        docT = _build_doc_ext(doc_reps[sl][order], ids_slab[order])
        qchE = np.zeros((K_EXT, TE * 128), dtype=qT.dtype)
        qchO = np.zeros((K_EXT, TO * 128), dtype=qT.dtype)
        # per-slot q columns, then scatter into even/odd chunk streams
        slotcols = {}
        for i in range(BD_PER):
            L = lists[order[i]]
            cols = np.zeros((K_EXT, pattern[i] * 128), dtype=qT.dtype)
            cols[:, : len(L)] = qT[:, L]
            b = np.full(pattern[i] * 128, -1, dtype=np.int64)
            b[: len(L)] = qpos_b[L]
            slotcols[i] = (cols, b)
        colb_parts = []
        for grp in groups:
            for dst, base, lst in (
                (qchE, grp["e0"], grp["ev"]),
                (qchO, grp["o0"], grp["od"]),
            ):
                for j, (slot, k) in enumerate(lst):
                    c = base + j
                    dst[:, c * 128 : (c + 1) * 128] = slotcols[slot][0][
                        :, k * 128 : (k + 1) * 128
                    ]
            # psum order: evens then odds
            for slot, k in grp["ev"] + grp["od"]:
                colb_parts.append(
                    (slot, slotcols[slot][1][k * 128 : (k + 1) * 128])
                )
        in_maps.append({"qchE": qchE, "qchO": qchO, "docT": docT})
        metas.append({"order": order, "colb_parts": colb_parts})
    meta = {"pattern": pattern, "groups": groups, "TE": TE, "TO": TO, "cores": metas}
    return in_maps, meta


def _split_multi_waits(nc, mybir):
    """walrus accepts one sync-wait per instruction; hoist extras into
    standalone EventSemaphore instructions on the same engine."""
    n = 0
    for func in nc.m.functions:
        for bb in func.blocks:
            out = []
            for inst in bb.instructions:
                si = inst.sync_info
                if si is not None and len(si.on_wait) > 1:
                    waits = list(si.on_wait)
                    for w in waits[:-1]:
                        n += 1
                        out.append(
                            mybir.InstEventSemaphore(
                                name=f"W-{inst.name}-{n}",
                                engine=inst.engine,
                                ins=[],
                                outs=[],
                                debug=inst.debug,
                                sync_info=mybir.SyncInfo(on_wait=[w], on_update=[]),
                            )
                        )
                    inst.sync_info = mybir.SyncInfo(
                        on_wait=[waits[-1]], on_update=list(si.on_update)
                    )
                out.append(inst)
            bb.instructions = out
    return n


def _build_nc(pattern, direct_groups, warmup):
    import concourse.bass as bass
    import concourse.mybir as mybir
    import concourse.tile as tile
    from concourse.bass import ts

    f8, f16, f32 = mybir.dt.float8e3, mybir.dt.float16, mybir.dt.float32
    groups, TE, TO = _layout(pattern)
    T = TE + TO
    ngrp = len(groups)
    sc_groups = [g for g in range(ngrp) if g not in direct_groups]
    sc_cols = {}
    off = 0
    for g in sc_groups:
        cg = groups[g]["cg"]
        sc_cols[g] = (off, off + cg)
        off += cg
    sc_total = off
    half = (len(sc_groups) + 1) // 2
    red1 = sc_groups[:half]
    red2 = sc_groups[half:]
    # group chunk-range starts in accA (psum order, group-major)
    acc0 = np.concatenate([[0], np.cumsum([g["cg"] for g in groups])]).astype(int)

    nc = bass.Bass("TRN2", target_bir_lowering=False, debug=False)
    qchE = nc.dram_tensor("qchE", [K_EXT, TE * 128], f8, kind="ExternalInput").ap()
    qchO = nc.dram_tensor("qchO", [K_EXT, TO * 128], f8, kind="ExternalInput").ap()
    docT = nc.dram_tensor("docT", [K_EXT, BD_PER * Sd], f8, kind="ExternalInput").ap()
    outA = nc.dram_tensor("outA", [128, T], f32, kind="ExternalOutput").ap()
    outB = nc.dram_tensor(
        "outB", [128, max(sc_total, 1)], f16, kind="ExternalOutput"
    ).ap()

    with tile.TileContext(nc) as tc:
        with (
            tc.tile_pool(name="inp", bufs=1) as inp,
            tc.tile_pool(name="psum", bufs=2, space="PSUM") as psum,
            tc.tile_pool(name="stg", bufs=1) as stg,
            tc.tile_pool(name="accp", bufs=1) as accp,
        ):
            qchE_sb = inp.tile([K_EXT, TE * 128], f8)
            qchO_sb = inp.tile([128, TO * 128], f8)  # data at partitions 64+
            docT_sb = inp.tile([128, BD_PER * Sd], f8)  # both row halves

            # input DMA spread over the three DGE queues in consumption order
            nc.gpsimd.dma_start(docT_sb[0:K_EXT, :, ], docT[:])
            nc.gpsimd.dma_start(docT_sb[64 : 64 + K_EXT, :], docT[:])
            he = TE * 128 // 2 // 128 * 128
            ho = TO * 128 // 2 // 128 * 128
            nc.sync.dma_start(qchE_sb[:, 0:he], qchE[:, 0:he])
            nc.scalar.dma_start(qchO_sb[64 : 64 + K_EXT, 0:ho], qchO[:, 0:ho])
            nc.sync.dma_start(qchE_sb[:, he : TE * 128], qchE[:, he : TE * 128])
            nc.scalar.dma_start(
                qchO_sb[64 : 64 + K_EXT, ho : TO * 128], qchO[:, ho : TO * 128]
            )

            # PE warm-up junk matmuls during the DMA head (HAM clock gate)
            scratch = inp.tile([K_EXT, 512], f8)
            nc.vector.memset(scratch[:], 0.0)
            wps = psum.tile([128, 512], f32, tag="warm")
            for _ in range(warmup):
                nc.tensor.matmul(
                    wps[:], scratch[:, 0:128], scratch[:], start=True, stop=True
                )

            accA = accp.tile([128, T], f32)
            staged = stg.tile([128, max(sc_total, 1) * 128], f16)
            n1 = sum(groups[g]["cg"] for g in red1)
            n2 = sc_total - n1
            accB1 = accp.tile([128, max(n1, 1)], f16)
            accB2 = accp.tile([128, max(n2, 1)], f16)
            negoff = accp.tile([128, 1], f32)
            nc.vector.memset(negoff[:], -OFF)

            for g, grp in enumerate(groups):
                cg = grp["cg"]
                ne = len(grp["ev"])
                ps = psum.tile([128, cg * 128], f32, tag="score")
                # row-tiled pairs: even chunk j at rows 0-61, odd at 64-125
                for j in range(ne):
                    eslot = grp["ev"][j][0]
                    nc.tensor.matmul(
                        ps[:, ts(j, 128)],
                        qchE_sb[:, ts(grp["e0"] + j, 128)],
                        docT_sb[0:K_EXT, ts(eslot, 128)],
                        start=True,
                        stop=True,
                        tile_position=(0, 0),
                    )
                    if j < len(grp["od"]):
                        oslot = grp["od"][j][0]
                        nc.tensor.matmul(
                            ps[:, ts(ne + j, 128)],
                            qchO_sb[64 : 64 + K_EXT, ts(grp["o0"] + j, 128)],
                            docT_sb[64 : 64 + K_EXT, ts(oslot, 128)],
                            start=True,
                            stop=True,
                            tile_position=(64, 0),
                        )
                c0, c1 = int(acc0[g]), int(acc0[g + 1])
                if g in direct_groups:
                    nc.vector.reduce_max(
                        accA[:, c0:c1],
                        ps[:].rearrange("p (c t) -> p c t", t=Sd),
                        axis=mybir.AxisListType.X,
                    )
                    nc.gpsimd.dma_start(outA[:, c0:c1], accA[:, c0:c1])
                else:
                    s0, s1 = sc_cols[g]
                    nc.scalar.activation(
                        staged[:, s0 * 128 : s1 * 128],
                        ps[:],
                        mybir.ActivationFunctionType.Relu,
                        bias=negoff[:],
                    )
                    if g == red1[-1]:
                        r0 = sc_cols[red1[0]][0]
                        r1 = sc_cols[red1[-1]][1]
                        nc.vector.reduce_max(
                            accB1[:],
                            staged[:, r0 * 128 : r1 * 128].rearrange(
                                "p (c t) -> p c t", t=Sd
                            ),
                            axis=mybir.AxisListType.X,
                        )
                        nc.gpsimd.dma_start(outB[:, 0:n1], accB1[:])
            if red2:
                r0 = sc_cols[red2[0]][0]
                r1 = sc_cols[red2[-1]][1]
                nc.vector.reduce_max(
                    accB2[:],
                    staged[:, r0 * 128 : r1 * 128].rearrange("p (c t) -> p c t", t=Sd),
                    axis=mybir.AxisListType.X,
                )
                # last-produced output: split rows across queues
                nc.sync.dma_start(outB[0:43, n1:sc_total], accB2[0:43, :])
                nc.scalar.dma_start(outB[43:86, n1:sc_total], accB2[43:86, :])
                nc.gpsimd.dma_start(outB[86:128, n1:sc_total], accB2[86:128, :])
    _split_multi_waits(nc, mybir)
    return nc


def _get_nc(pattern):
    key = (tuple(pattern), DIRECT_GROUPS, WARMUP_MMS)
    if key not in _CACHE:
        _CACHE[key] = _build_nc(tuple(pattern), DIRECT_GROUPS, WARMUP_MMS)
    return _CACHE[key]


def assemble(inputs, results, meta):
    pattern, groups = meta["pattern"], meta["groups"]
    ngrp = len(groups)
    sc_groups = [g for g in range(ngrp) if g not in DIRECT_GROUPS]
    acc0 = np.concatenate([[0], np.cumsum([g["cg"] for g in groups])]).astype(int)
    toks = np.zeros((Bq, Bd), dtype=np.float32)
    for core in range(NCORES):
        m = meta["cores"][core]
        outA = np.asarray(results[core]["outA"], np.float32)  # [128, T]
        tok = np.maximum(outA, OFF) - OFF
        if sc_groups:
            outB = np.asarray(results[core]["outB"], np.float32)
            off = 0
            for g in sc_groups:
                c0, c1 = int(acc0[g]), int(acc0[g + 1])
                tok[:, c0:c1] = outB[:, off : off + (c1 - c0)]
                off += c1 - c0
        order = np.asarray(m["order"])
        for c, (slot, b) in enumerate(m["colb_parts"]):
            ok = b >= 0
            docidx = core * BD_PER + order[slot]
            np.add.at(toks[:, docidx], b[ok], tok[ok, c])
    cls = np.asarray(inputs["qry_cls"], np.float32) @ np.asarray(
        inputs["doc_cls"], np.float32
    ).T
    return (toks + cls).max(axis=0).reshape(-1).astype(np.float32)


def _ensure_ntff_hook():
    import sys
    import types

    if "antenv.axon_hooks" in sys.modules:
        return
    mod = types.ModuleType("antenv.axon_hooks")
    state = {"hook": None}
    mod.set_axon_ntff_profile_hook = lambda h: state.__setitem__("hook", h)
    mod.get_axon_ntff_profile_hook = lambda: state["hook"]
    sys.modules["antenv.axon_hooks"] = mod
    try:
        import antenv

        antenv.axon_hooks = mod
    except ImportError:
        pass
    try:
        from trn_agent_boot.trn_boot import _ntff_profile_via_ctypes

        mod.set_axon_ntff_profile_hook(
            _ntff_profile_via_ctypes("/opt/axon/libaxon_pjrt.so")
        )
    except Exception:
        pass


def run(inputs, trace=False, **kwargs):
    from concourse.bass_utils import run_bass_kernel_spmd

    if trace:
        _ensure_ntff_hook()
    in_maps, meta = prepare(inputs)
    nc = _get_nc(meta["pattern"])
    res = run_bass_kernel_spmd(
        nc, in_maps, core_ids=list(range(NCORES)), trace=trace, **kwargs
    )
    return assemble(inputs, res.results, meta), res


def kernel(**inputs) -> np.ndarray:
    out, _ = run(inputs)
    return out


# revision 3
# speedup vs baseline: 1.2138x; 1.1257x over previous
"""COIL sparse-attention scoring kernel v2: per-doc q-compacted chunks.

Strategy
--------
Shard docs 16/core. A query row can only contribute to a doc containing its
token id (~12% of (q,doc) pairs; host knows this from ids alone). For each
(core, doc) the host compacts the matching query rows into 128-slot chunks;
the device runs one [62,128]x[62,128] bf16 matmul per chunk (K = 32 rep dims
+ 30 digit one-hot dims so a full id match adds +3072 to the score) and
reduces max over the doc's 128 tokens per slot. This cuts PSUM volume ~7x
vs the dense [all-q x all-doc-tokens] formulation.

Docs are sorted by need and slot-assigned so the per-slot chunk counts
(pattern) are consistent across cores. Chunks are grouped in slot pairs into
[128, cg*128] PSUM tiles. K=62 <= 64 lets two chunks run CONCURRENTLY as
row-tiled matmuls (tile_position (0,0) / (64,0)): even chunks live in SBUF
partitions 0-61, odd chunks in 64-125 (docT replicated to both halves), so
each LDWEIGHTS overlaps the other row-half's matmul and the PE streams at
~1 chunk per 107ns even cold. Groups are consumed either by DVE reduce_max
straight from PSUM (raw max; host relu-shifts) or ScalarE relu(v-OFF) into
fp16 staging with batched DVE reduce_max at 2x rate. Input/output DMA is
spread over the three DGE queues (sync/scalar/gpsimd, ~40-50 GB/s each) in
consumption order. Host scatters per-slot tok values back to [Bq, Bd], adds
cls scores, maxes over Bq.
"""

import os
import numpy as np
import ml_dtypes

Bq, Sq, Bd, Sd, D = 8, 512, 128, 128, 32
NCORES = 8
BD_PER = Bd // NCORES
SQF = Bq * Sq
# fp8 e3m4 inputs (4-bit mantissa, max 15.5): reps fit (|x|<5.2), digit
# one-hots at ALPHA=12 are exact, and input DMA bytes halve vs bf16 — the
# binding constraint is HBM bandwidth shared across all 8 cores (~40GB/s
# per core when all load simultaneously).
K_EXT = 56  # 32 rep dims + 24 base-6 digit one-hot dims
ALPHA = 12.0
NDIGITS = 4
OFF = NDIGITS * ALPHA * ALPHA  # 576
WARMUP_MMS = int(os.environ.get("KERNEL_WARMUP_MMS", "10"))
DIRECT_GROUPS = tuple(
    int(x)
    for x in os.environ.get("KERNEL_DIRECT_GROUPS", "0,3,7").split(",")
    if x != ""
)

_CACHE = {}


def _fp8(x):
    return x.astype(ml_dtypes.float8_e3m4)


def _onehot_digits(ids):
    """base-6 4-digit one-hot (ids < 1000 < 6^4): [N, 24]."""
    n = ids.shape[0]
    H = np.zeros((n, 24), dtype=np.float32)
    r = np.arange(n)
    H[r, ids % 6] = 1.0
    H[r, 6 + (ids // 6) % 6] = 1.0
    H[r, 12 + (ids // 36) % 6] = 1.0
    H[r, 18 + ids // 216] = 1.0
    return H


def _qry_row_mask(inputs):
    mask = np.asarray(inputs["qry_attention_mask"], np.int64).copy()
    sep = mask.sum(axis=1) - 1
    mask[np.arange(Bq), sep] = 0
    mask[:, 0] = 0
    return mask.astype(bool)


def _build_qry_ext(inputs):
    q = np.asarray(inputs["qry_reps"], np.float32).reshape(SQF, D)
    ids = np.asarray(inputs["qry_input_ids"], np.int64).reshape(SQF)
    ext = np.concatenate(
        [_fp8(q).astype(np.float32), ALPHA * _onehot_digits(ids)], axis=1
    )
    ext *= _qry_row_mask(inputs).reshape(SQF, 1)
    return np.ascontiguousarray(_fp8(ext).T)  # [56, SQF]


def _build_doc_ext(doc_reps, doc_ids):
    d = np.asarray(doc_reps, np.float32).reshape(-1, D)
    ids = np.asarray(doc_ids, np.int64).reshape(-1)
    ext = np.concatenate(
        [_fp8(d).astype(np.float32), ALPHA * _onehot_digits(ids)], axis=1
    )
    return np.ascontiguousarray(_fp8(ext).T)  # [56, N]


def _layout(pattern):
    """Chunk layout derived purely from the slot pattern (shared by host and
    device builder). Returns per-group dicts:
      chunks: list of (slot, k) in PSUM column order (evens then odds)
      ne/no: even/odd counts; e0/o0: column offsets into qchE/qchO
    plus global chunk-id order (group-major, psum order) for output mapping.
    """
    groups = []
    e_off = o_off = 0
    for g in range(BD_PER // 2):
        s0, s1 = 2 * g, 2 * g + 1
        seq = [(s0, k) for k in range(pattern[s0])] + [
            (s1, k) for k in range(pattern[s1])
        ]
        ev, od = seq[0::2], seq[1::2]
        groups.append(
            {
                "slots": (s0, s1),
                "ev": ev,
                "od": od,
                "e0": e_off,
                "o0": o_off,
                "cg": len(seq),
            }
        )
        e_off += len(ev)
        o_off += len(od)
    return groups, e_off, o_off


def prepare(inputs):
    qT = _build_qry_ext(inputs)
    doc_reps = np.asarray(inputs["doc_reps"], np.float32)
    doc_ids = np.asarray(inputs["doc_input_ids"], np.int64)
    qry_ids = np.asarray(inputs["qry_input_ids"], np.int64).reshape(SQF)
    row_ok = _qry_row_mask(inputs).reshape(SQF)
    qpos_b = np.repeat(np.arange(Bq), Sq)

    cores = []
    for core in range(NCORES):
        sl = slice(core * BD_PER, (core + 1) * BD_PER)
        ids_slab = doc_ids[sl]
        lists = []
        for d in range(BD_PER):
            vocab = np.zeros(1000, dtype=bool)
            vocab[ids_slab[d]] = True
            lists.append(np.nonzero(row_ok & vocab[qry_ids])[0])
        bysize = np.argsort([-len(L) for L in lists], kind="stable")
        order = np.empty(BD_PER, dtype=np.int64)
        order[0::2] = bysize[: BD_PER // 2]
        order[1::2] = bysize[BD_PER // 2 :][::-1]
        cores.append((sl, ids_slab, lists, order))

    pattern = tuple(
        max(max((len(c[2][c[3][i]]) + 127) // 128 for c in cores), 1)
        for i in range(BD_PER)
    )
    groups, TE, TO = _layout(pattern)

    in_maps, metas = [], []
    for core in range(NCORES):
        sl, ids_slab, lists, order = cores[core]
        docT = _build_doc_ext(doc_reps[sl][order], ids_slab[order])
        qchE = np.zeros((K_EXT, TE * 128), dtype=qT.dtype)
        qchO = np.zeros((K_EXT, TO * 128), dtype=qT.dtype)
        # per-slot q columns, then scatter into even/odd chunk streams
        slotcols = {}
        for i in range(BD_PER):
            L = lists[order[i]]
            cols = np.zeros((K_EXT, pattern[i] * 128), dtype=qT.dtype)
            cols[:, : len(L)] = qT[:, L]
            b = np.full(pattern[i] * 128, -1, dtype=np.int64)
            b[: len(L)] = qpos_b[L]
            slotcols[i] = (cols, b)
        colb_parts = []
        for grp in groups:
            for dst, base, lst in (
                (qchE, grp["e0"], grp["ev"]),
                (qchO, grp["o0"], grp["od"]),
            ):
                for j, (slot, k) in enumerate(lst):
                    c = base + j
                    dst[:, c * 128 : (c + 1) * 128] = slotcols[slot][0][
                        :, k * 128 : (k + 1) * 128
                    ]
            # psum order: evens then odds
            for slot, k in grp["ev"] + grp["od"]:
                colb_parts.append(
                    (slot, slotcols[slot][1][k * 128 : (k + 1) * 128])
                )
        in_maps.append({"qchE": qchE, "qchO": qchO, "docT": docT})
        metas.append({"order": order, "colb_parts": colb_parts})
    meta = {"pattern": pattern, "groups": groups, "TE": TE, "TO": TO, "cores": metas}
    return in_maps, meta


def _split_multi_waits(nc, mybir):
    """walrus accepts one sync-wait per instruction; hoist extras into
    standalone EventSemaphore instructions on the same engine."""
    n = 0
    for func in nc.m.functions:
        for bb in func.blocks:
            out = []
            for inst in bb.instructions:
                si = inst.sync_info
                if si is not None and len(si.on_wait) > 1:
                    waits = list(si.on_wait)
                    for w in waits[:-1]:
                        n += 1
                        out.append(
                            mybir.InstEventSemaphore(
                                name=f"W-{inst.name}-{n}",
                                engine=inst.engine,
                                ins=[],
                                outs=[],
                                debug=inst.debug,
                                sync_info=mybir.SyncInfo(on_wait=[w], on_update=[]),
                            )
                        )
                    inst.sync_info = mybir.SyncInfo(
                        on_wait=[waits[-1]], on_update=list(si.on_update)
                    )
                out.append(inst)
            bb.instructions = out
    return n


def _build_nc(pattern, direct_groups, warmup):
    import concourse.bass as bass
    import concourse.mybir as mybir
    import concourse.tile as tile
    from concourse.bass import ts

    f8, f16, f32 = mybir.dt.float8e3, mybir.dt.float16, mybir.dt.float32
    groups, TE, TO = _layout(pattern)
    T = TE + TO
    ngrp = len(groups)
    sc_groups = [g for g in range(ngrp) if g not in direct_groups]
    sc_cols = {}
    off = 0
    for g in sc_groups:
        cg = groups[g]["cg"]
        sc_cols[g] = (off, off + cg)
        off += cg
    sc_total = off
    # group chunk-range starts in accA (psum order, group-major)
    acc0 = np.concatenate([[0], np.cumsum([g["cg"] for g in groups])]).astype(int)

    nc = bass.Bass("TRN2", target_bir_lowering=False, debug=False)
    qchE = nc.dram_tensor("qchE", [K_EXT, TE * 128], f8, kind="ExternalInput").ap()
    qchO = nc.dram_tensor("qchO", [K_EXT, TO * 128], f8, kind="ExternalInput").ap()
    docT = nc.dram_tensor("docT", [K_EXT, BD_PER * Sd], f8, kind="ExternalInput").ap()
    outA = nc.dram_tensor("outA", [128, T], f32, kind="ExternalOutput").ap()
    outB = nc.dram_tensor(
        "outB", [128, max(sc_total, 1)], f16, kind="ExternalOutput"
    ).ap()

    with tile.TileContext(nc) as tc:
        with (
            tc.tile_pool(name="inp", bufs=1) as inp,
            tc.tile_pool(name="psum", bufs=2, space="PSUM") as psum,
            tc.tile_pool(name="stg", bufs=1) as stg,
            tc.tile_pool(name="accp", bufs=1) as accp,
        ):
            qchE_sb = inp.tile([K_EXT, TE * 128], f8)
            qchO_sb = inp.tile([128, TO * 128], f8)  # data at partitions 64+
            docT_sb = inp.tile([128, BD_PER * Sd], f8)  # both row halves

            # input DMA: one full-width transfer per tensor (maximizes the
            # per-partition-row packet size; the DGE is packet-rate-bound),
            # spread across the three DGE queues
            nc.gpsimd.dma_start(docT_sb[0:K_EXT, :, ], docT[:])
            nc.gpsimd.dma_start(docT_sb[64 : 64 + K_EXT, :], docT[:])
            nc.sync.dma_start(qchE_sb[:], qchE[:])
            nc.scalar.dma_start(qchO_sb[64 : 64 + K_EXT, :], qchO[:])

            # PE warm-up junk matmuls during the DMA head (HAM clock gate)
            scratch = inp.tile([K_EXT, 512], f8)
            nc.vector.memset(scratch[:], 0.0)
            wps = psum.tile([128, 512], f32, tag="warm")
            for _ in range(warmup):
                nc.tensor.matmul(
                    wps[:], scratch[:, 0:128], scratch[:], start=True, stop=True
                )

            accA = accp.tile([128, T], f32)
            staged = stg.tile([128, max(sc_total, 1) * 128], f16)
            accB = accp.tile([128, max(sc_total, 1)], f16)
            negoff = accp.tile([128, 1], f32)
            nc.vector.memset(negoff[:], -OFF)

            for g, grp in enumerate(groups):
                cg = grp["cg"]
                ne = len(grp["ev"])
                ps = psum.tile([128, cg * 128], f32, tag="score")
                # row-tiled pairs: even chunk j at rows 0-61, odd at 64-125
                for j in range(ne):
                    eslot = grp["ev"][j][0]
                    nc.tensor.matmul(
                        ps[:, ts(j, 128)],
                        qchE_sb[:, ts(grp["e0"] + j, 128)],
                        docT_sb[0:K_EXT, ts(eslot, 128)],
                        start=True,
                        stop=True,
                        tile_position=(0, 0),
                    )
                    if j < len(grp["od"]):
                        oslot = grp["od"][j][0]
                        nc.tensor.matmul(
                            ps[:, ts(ne + j, 128)],
                            qchO_sb[64 : 64 + K_EXT, ts(grp["o0"] + j, 128)],
                            docT_sb[64 : 64 + K_EXT, ts(oslot, 128)],
                            start=True,
                            stop=True,
                            tile_position=(64, 0),
                        )
                c0, c1 = int(acc0[g]), int(acc0[g + 1])
                if g in direct_groups:
                    nc.vector.reduce_max(
                        accA[:, c0:c1],
                        ps[:].rearrange("p (c t) -> p c t", t=Sd),
                        axis=mybir.AxisListType.X,
                    )
                    nc.gpsimd.dma_start(outA[:, c0:c1], accA[:, c0:c1])
                else:
                    # per-group staged reduce: fires right after this group's
                    # activation so the DVE work overlaps the MM stream
                    # instead of piling into a serial tail (TENSOR_REDUCE
                    # runs at 1x regardless of dtype in this stack)
                    s0, s1 = sc_cols[g]
                    nc.scalar.activation(
                        staged[:, s0 * 128 : s1 * 128],
                        ps[:],
                        mybir.ActivationFunctionType.Relu,
                        bias=negoff[:],
                    )
                    nc.vector.reduce_max(
                        accB[:, s0:s1],
                        staged[:, s0 * 128 : s1 * 128].rearrange(
                            "p (c t) -> p c t", t=Sd
                        ),
                        axis=mybir.AxisListType.X,
                    )
                    if g == sc_groups[-1]:
                        nc.sync.dma_start(outB[:], accB[:])
    _split_multi_waits(nc, mybir)
    return nc


def _get_nc(pattern):
    key = (tuple(pattern), DIRECT_GROUPS, WARMUP_MMS)
    if key not in _CACHE:
        _CACHE[key] = _build_nc(tuple(pattern), DIRECT_GROUPS, WARMUP_MMS)
    return _CACHE[key]


def assemble(inputs, results, meta):
    pattern, groups = meta["pattern"], meta["groups"]
    ngrp = len(groups)
    sc_groups = [g for g in range(ngrp) if g not in DIRECT_GROUPS]
    acc0 = np.concatenate([[0], np.cumsum([g["cg"] for g in groups])]).astype(int)
    toks = np.zeros((Bq, Bd), dtype=np.float32)
    for core in range(NCORES):
        m = meta["cores"][core]
        outA = np.asarray(results[core]["outA"], np.float32)  # [128, T]
        tok = np.maximum(outA, OFF) - OFF
        if sc_groups:
            outB = np.asarray(results[core]["outB"], np.float32)
            off = 0
            for g in sc_groups:
                c0, c1 = int(acc0[g]), int(acc0[g + 1])
                tok[:, c0:c1] = outB[:, off : off + (c1 - c0)]
                off += c1 - c0
        order = np.asarray(m["order"])
        for c, (slot, b) in enumerate(m["colb_parts"]):
            ok = b >= 0
            docidx = core * BD_PER + order[slot]
            np.add.at(toks[:, docidx], b[ok], tok[ok, c])
    cls = np.asarray(inputs["qry_cls"], np.float32) @ np.asarray(
        inputs["doc_cls"], np.float32
    ).T
    return (toks + cls).max(axis=0).reshape(-1).astype(np.float32)


def _ensure_ntff_hook():
    import sys
    import types

    if "antenv.axon_hooks" in sys.modules:
        return
    mod = types.ModuleType("antenv.axon_hooks")
    state = {"hook": None}
    mod.set_axon_ntff_profile_hook = lambda h: state.__setitem__("hook", h)
    mod.get_axon_ntff_profile_hook = lambda: state["hook"]
    sys.modules["antenv.axon_hooks"] = mod
    try:
        import antenv

        antenv.axon_hooks = mod
    except ImportError:
        pass
    try:
        from trn_agent_boot.trn_boot import _ntff_profile_via_ctypes

        mod.set_axon_ntff_profile_hook(
            _ntff_profile_via_ctypes("/opt/axon/libaxon_pjrt.so")
        )
    except Exception:
        pass


def run(inputs, trace=False, **kwargs):
    from concourse.bass_utils import run_bass_kernel_spmd

    if trace:
        _ensure_ntff_hook()
    in_maps, meta = prepare(inputs)
    nc = _get_nc(meta["pattern"])
    res = run_bass_kernel_spmd(
        nc, in_maps, core_ids=list(range(NCORES)), trace=trace, **kwargs
    )
    return assemble(inputs, res.results, meta), res


def kernel(**inputs) -> np.ndarray:
    out, _ = run(inputs)
    return out


# revision 5
# speedup vs baseline: 1.4472x; 1.1923x over previous
"""COIL kernel v3: global id-sorted query tiles + per-tile gathered doc tokens.

Queries (all attended rows, id-sorted, 32 tiles of 128) are sent ONCE per
core. For each (tile, doc) the host gathers the <=8 doc tokens whose id
appears in the tile (W=8 slots, zero-padded; overflow pairs corrected on
host). One [56,128]x[56,128] fp8 matmul per tile scores every query in the
tile against its candidate tokens for all 16 docs; DVE reduce-max over the
8-token windows gives tok per (q,doc). This removes the per-doc query
duplication of v2: input drops to ~0.46MB/core, 32 matmuls, PSUM 0.5M f32.
"""

import os
import numpy as np
import ml_dtypes

Bq, Sq, Bd, Sd, D = 8, 512, 128, 128, 32
NCORES = 8
BD_PER = Bd // NCORES
SQF = Bq * Sq
K_EXT = 56
ALPHA = 12.0
NDIGITS = 4
OFF = NDIGITS * ALPHA * ALPHA  # 576
W = 8                      # candidate doc-token slots per (tile, doc)
NT = 32                    # query tiles
GRP = 8                    # tiles per PSUM group
WARMUP_MMS = int(os.environ.get("KERNEL_WARMUP_MMS", "10"))

_CACHE = {}


def _fp8(x):
    return x.astype(ml_dtypes.float8_e3m4)


def _onehot_digits(ids):
    n = ids.shape[0]
    H = np.zeros((n, 24), dtype=np.float32)
    r = np.arange(n)
    H[r, ids % 6] = 1.0
    H[r, 6 + (ids // 6) % 6] = 1.0
    H[r, 12 + (ids // 36) % 6] = 1.0
    H[r, 18 + ids // 216] = 1.0
    return H


def _qry_row_mask(inputs):
    mask = np.asarray(inputs["qry_attention_mask"], np.int64).copy()
    sep = mask.sum(axis=1) - 1
    mask[np.arange(Bq), sep] = 0
    mask[:, 0] = 0
    return mask.astype(bool)


def _ext(reps, ids):
    out = np.concatenate(
        [_fp8(reps).astype(np.float32), ALPHA * _onehot_digits(ids)], axis=1
    )
    return _fp8(out)  # [N, 56]


def prepare(inputs):
    q = np.asarray(inputs["qry_reps"], np.float32).reshape(SQF, D)
    qry_ids = np.asarray(inputs["qry_input_ids"], np.int64).reshape(SQF)
    row_ok = _qry_row_mask(inputs).reshape(SQF)
    doc_reps = np.asarray(inputs["doc_reps"], np.float32)
    doc_ids = np.asarray(inputs["doc_input_ids"], np.int64)
    qpos_b = np.repeat(np.arange(Bq), Sq)

    rows = np.nonzero(row_ok)[0]
    rows = rows[np.argsort(qry_ids[rows], kind="stable")]
    nrow = len(rows)
    assert nrow <= NT * 128
    qe = np.zeros((NT * 128, K_EXT), dtype=ml_dtypes.float8_e3m4)
    qe[:nrow] = _ext(q[rows], qry_ids[rows])
    qT = np.ascontiguousarray(qe.T)               # [56, 4096] fp8
    colb = np.full(NT * 128, -1, dtype=np.int64)
    colb[:nrow] = qpos_b[rows]
    tile_ids = [set(qry_ids[rows[t * 128 : (t + 1) * 128]].tolist())
                for t in range(NT)]
    tile_ids = [tile_ids[t] if t * 128 < nrow else set() for t in range(NT)]

    # split even/odd tiles for the two row-tiled halves
    qTE = np.ascontiguousarray(
        qT.reshape(K_EXT, NT, 128)[:, 0::2].reshape(K_EXT, NT // 2 * 128)
    )
    qTO = np.ascontiguousarray(
        qT.reshape(K_EXT, NT, 128)[:, 1::2].reshape(K_EXT, NT // 2 * 128)
    )

    in_maps, metas = [], []
    for core in range(NCORES):
        sl = slice(core * BD_PER, (core + 1) * BD_PER)
        dreps = doc_reps[sl].reshape(-1, D)
        dids = doc_ids[sl]
        de = _ext(dreps, dids.reshape(-1)).astype(np.float32)  # [2048, 56]
        docG = np.zeros((NT * 128, K_EXT), dtype=np.float32)
        overflow = []                      # (tile, doc, token_global_idx)
        for t in range(NT):
            ids_t = tile_ids[t]
            for d in range(BD_PER):
                tok_idx = [d * Sd + k for k in range(Sd) if dids[d, k] in ids_t]
                base = t * 128 + d * W
                take = tok_idx[:W]
                docG[base : base + len(take)] = de[take]
                for ov in tok_idx[W:]:
                    overflow.append((t, d, ov))
        docGf = _fp8(docG).T               # [56, 4096]
        docGE = np.ascontiguousarray(
            docGf.reshape(K_EXT, NT, 128)[:, 0::2].reshape(K_EXT, NT // 2 * 128)
        )
        docGO = np.ascontiguousarray(
            docGf.reshape(K_EXT, NT, 128)[:, 1::2].reshape(K_EXT, NT // 2 * 128)
        )
        in_maps.append({"qTE": qTE, "qTO": qTO, "docGE": docGE, "docGO": docGO})
        metas.append({"overflow": overflow, "de": de, "dids": dids})
    ids_sorted = np.full(NT * 128, -1, dtype=np.int64)
    ids_sorted[:nrow] = qry_ids[rows]
    meta = {
        "colb": colb,
        "ids_sorted": ids_sorted,
        "qT": qT,
        "cores": metas,
    }
    return in_maps, meta


def _tile_slot(t):
    """outT column slot for tile t (evens first half of each group's PSUM)."""
    g, i = t // GRP, t % GRP
    s = i // 2 if i % 2 == 0 else GRP // 2 + i // 2
    return g * GRP + s


def assemble(inputs, results, meta):
    colb = meta["colb"]
    ids_sorted = meta["ids_sorted"]
    qT = np.asarray(meta["qT"], np.float32)     # [56, 4096]
    toks = np.zeros((Bq, Bd), dtype=np.float32)
    for core in range(NCORES):
        m = meta["cores"][core]
        tok = np.asarray(results[core]["outT"], np.float32)  # [128, NT*16]
        # overflow corrections: device kept only the first W matching tokens
        for (t, d, ov) in m["overflow"]:
            tok_id = m["dids"][ov // Sd, ov % Sd]
            seg = slice(t * 128, (t + 1) * 128)
            qcols = np.nonzero(ids_sorted[seg] == tok_id)[0]
            if len(qcols) == 0:
                continue
            # v includes the full digit-match offset (id_q == tok_id)
            v = qT[:, t * 128 + qcols].T @ np.asarray(m["de"][ov], np.float32)
            contrib = np.maximum(v - OFF, 0.0)
            sc = _tile_slot(t) * 16 + d
            tok[qcols, sc] = np.maximum(tok[qcols, sc], contrib)
        for t in range(NT):
            seg = slice(t * 128, (t + 1) * 128)
            bseg = colb[seg]
            good = bseg >= 0
            if not good.any():
                continue
            np.add.at(
                toks[:, core * BD_PER : (core + 1) * BD_PER],
                (bseg[good], slice(None)),
                tok[good, _tile_slot(t) * 16 : (_tile_slot(t) + 1) * 16],
            )
    cls = np.asarray(inputs["qry_cls"], np.float32) @ np.asarray(
        inputs["doc_cls"], np.float32
    ).T
    return (toks + cls).max(axis=0).reshape(-1).astype(np.float32)


def _split_multi_waits(nc, mybir):
    n = 0
    for func in nc.m.functions:
        for bb in func.blocks:
            out = []
            for inst in bb.instructions:
                si = inst.sync_info
                if si is not None and len(si.on_wait) > 1:
                    waits = list(si.on_wait)
                    for w in waits[:-1]:
                        n += 1
                        out.append(
                            mybir.InstEventSemaphore(
                                name=f"W-{inst.name}-{n}",
                                engine=inst.engine,
                                ins=[],
                                outs=[],
                                debug=inst.debug,
                                sync_info=mybir.SyncInfo(on_wait=[w], on_update=[]),
                            )
                        )
                    inst.sync_info = mybir.SyncInfo(
                        on_wait=[waits[-1]], on_update=list(si.on_update)
                    )
                out.append(inst)
            bb.instructions = out
    return n


def _build_nc(warmup):
    import concourse.bass as bass
    import concourse.mybir as mybir
    import concourse.tile as tile
    from concourse.bass import ts

    f8, f16, f32 = mybir.dt.float8e3, mybir.dt.float16, mybir.dt.float32
    NH = NT // 2
    nc = bass.Bass("TRN2", target_bir_lowering=False, debug=False)
    qTE = nc.dram_tensor("qTE", [K_EXT, NH * 128], f8, kind="ExternalInput").ap()
    qTO = nc.dram_tensor("qTO", [K_EXT, NH * 128], f8, kind="ExternalInput").ap()
    dGE = nc.dram_tensor("docGE", [K_EXT, NH * 128], f8, kind="ExternalInput").ap()
    dGO = nc.dram_tensor("docGO", [K_EXT, NH * 128], f8, kind="ExternalInput").ap()
    outT = nc.dram_tensor("outT", [128, NT * 16], f16, kind="ExternalOutput").ap()

    ngrp = NT // GRP
    with tile.TileContext(nc) as tc:
        with (
            tc.tile_pool(name="inp", bufs=1) as inp,
            tc.tile_pool(name="psum", bufs=2, space="PSUM") as psum,
            tc.tile_pool(name="accp", bufs=1) as accp,
        ):
            qTE_sb = inp.tile([K_EXT, NH * 128], f8)
            qTO_sb = inp.tile([128, NH * 128], f8)
            dGE_sb = inp.tile([K_EXT, NH * 128], f8)
            dGO_sb = inp.tile([128, NH * 128], f8)
            nc.sync.dma_start(qTE_sb[:], qTE[:])
            nc.scalar.dma_start(qTO_sb[64 : 64 + K_EXT, :], qTO[:])
            nc.gpsimd.dma_start(dGE_sb[:], dGE[:])
            nc.gpsimd.dma_start(dGO_sb[64 : 64 + K_EXT, :], dGO[:])

            scratch = inp.tile([K_EXT, 512], f8)
            nc.vector.memset(scratch[:], 0.0)
            wps = psum.tile([128, 512], f32, tag="warm")
            for _ in range(warmup):
                nc.tensor.matmul(
                    wps[:], scratch[:, 0:128], scratch[:], start=True, stop=True
                )

            accR = accp.tile([128, NT * 16], f32)
            accT = accp.tile([128, NT * 16], f16)
            negoff = accp.tile([128, 1], f32)
            nc.vector.memset(negoff[:], -OFF)

            for g in range(ngrp):
                ps = psum.tile([128, GRP * 128], f32, tag="score")
                # concurrent row-tiled pairs must drain into DIFFERENT PSUM
                # banks: even tiles fill the first half of the group tile,
                # odd tiles the second half (adjacent columns share a bank
                # and concurrent drains there hang the PE)
                for k in range(GRP // 2):
                    t = g * GRP + 2 * k
                    j = t // 2
                    nc.tensor.matmul(
                        ps[:, ts(k, 128)],
                        qTE_sb[:, ts(j, 128)],
                        dGE_sb[:, ts(j, 128)],
                        start=True,
                        stop=True,
                        tile_position=(0, 0),
                    )
                    nc.tensor.matmul(
                        ps[:, ts(GRP // 2 + k, 128)],
                        qTO_sb[64 : 64 + K_EXT, ts(j, 128)],
                        dGO_sb[64 : 64 + K_EXT, ts(j, 128)],
                        start=True,
                        stop=True,
                        tile_position=(64, 0),
                    )
                c0 = g * GRP * 16
                c1 = (g + 1) * GRP * 16
                nc.vector.reduce_max(
                    accR[:, c0:c1],
                    ps[:].rearrange("p (c w) -> p c w", w=W),
                    axis=mybir.AxisListType.X,
                )
                nc.scalar.activation(
                    accT[:, c0:c1],
                    accR[:, c0:c1],
                    mybir.ActivationFunctionType.Relu,
                    bias=negoff[:],
                )
            nc.sync.dma_start(outT[0:43, :], accT[0:43, :])
            nc.scalar.dma_start(outT[43:86, :], accT[43:86, :])
            nc.gpsimd.dma_start(outT[86:128, :], accT[86:128, :])
    _split_multi_waits(nc, mybir)
    return nc


def _get_nc():
    key = (WARMUP_MMS,)
    if key not in _CACHE:
        _CACHE[key] = _build_nc(WARMUP_MMS)
    return _CACHE[key]


def _ensure_ntff_hook():
    import sys
    import types

    if "antenv.axon_hooks" in sys.modules:
        return
    mod = types.ModuleType("antenv.axon_hooks")
    state = {"hook": None}
    mod.set_axon_ntff_profile_hook = lambda h: state.__setitem__("hook", h)
    mod.get_axon_ntff_profile_hook = lambda: state["hook"]
    sys.modules["antenv.axon_hooks"] = mod
    try:
        import antenv

        antenv.axon_hooks = mod
    except ImportError:
        pass
    try:
        from trn_agent_boot.trn_boot import _ntff_profile_via_ctypes

        mod.set_axon_ntff_profile_hook(
            _ntff_profile_via_ctypes("/opt/axon/libaxon_pjrt.so")
        )
    except Exception:
        pass


def run(inputs, trace=False, **kwargs):
    from concourse.bass_utils import run_bass_kernel_spmd

    if trace:
        _ensure_ntff_hook()
    in_maps, meta = prepare(inputs)
    nc = _get_nc()
    res = run_bass_kernel_spmd(
        nc, in_maps, core_ids=list(range(NCORES)), trace=trace, **kwargs
    )
    return assemble(inputs, res.results, meta), res


def kernel(**inputs) -> np.ndarray:
    out, _ = run(inputs)
    return out


def emulate_core(in_map):
    qTE = np.asarray(in_map["qTE"], np.float32)
    qTO = np.asarray(in_map["qTO"], np.float32)
    dGE = np.asarray(in_map["docGE"], np.float32)
    dGO = np.asarray(in_map["docGO"], np.float32)
    tokc = np.zeros((128, NT * 16), np.float32)
    for t in range(NT):
        src_q = qTE if t % 2 == 0 else qTO
        src_d = dGE if t % 2 == 0 else dGO
        j = t // 2
        st = src_q[:, j * 128 : (j + 1) * 128]
        mov = src_d[:, j * 128 : (j + 1) * 128]
        v = st.T @ mov                                 # [128 q, 128 = 16d*8w]
        raw = v.reshape(128, 16, W).max(axis=2)        # [128, 16]
        s = _tile_slot(t)
        tokc[:, s * 16 : (s + 1) * 16] = np.maximum(raw, OFF) - OFF
    return tokc


